# revision 32
# baseline (speedup 1.0000x reference)
# Multi-head attention (N=4, S=2048, E=512, H=8, D=64) on 8 NeuronCores.
#
# Sharding: core c -> (batch n = c//2, query half qh = c%2). Each core
# computes attention for its 1024 query rows against the (compacted) keys of
# its batch, all 8 heads, and the full output projection for its rows, so
# outputs are disjoint and no collectives are needed.
#
# Key compaction: the mask is per-(batch, key) broadcast over queries/heads,
# and masked keys get exp(-1.25e8) == 0 exactly in f32, contributing nothing
# to the numerator or denominator. So keys with mask==0 are dropped on the
# host; all batches pad to a common chunk-multiple length SKC with pad keys
# biased to -inf. This halves score/exp/attn@V work for ~50% masks.
#
# Host-side prep (pure weight algebra / data layout, done once):
#   A  = Wq^T @ Wk / 8          scores = (Xq @ A) @ Xk^T  (raw K, one proj)
#   M_h = Wv^T @ Wo[:, h]^T     out += (attn @ Xv_h) @ M_h (Wv applied post)
#   btot = bo + Wo @ tile(bv,8) exact because attention rows sum to 1
#   bk-term cancels in softmax (constant over k); bq-term handled via a
#   per-k bias correction (zero for this problem's inputs).
#   Q/K/V are cast to bf16 and pre-laid-out in the SBUF-resident shapes
#   ([e, s] transposes, V with its ones column), so on device they are
#   plain contiguous DMAs with chunk-granular dependencies.
#
# Device per core (head pair p owns heads 2p, 2p+1 at partition bases 0/64):
#   - xqa = blockdiag(A,A)-projection of Q^T per head pair (128-contraction)
#   - per (head, k-chunk): scores^T[k,q] on PE -> exp on ACT straight from
#     PSUM with per-partition mask bias -> bf16 ex tiles
#   - attn@V accumulates per chunk at 4-chunk lag (V carries a ones column
#     so softmax denominators ride along as PSUM row 64)
#   - normalization is split so no engine queue ever waits on its DMAs:
#     norm_start (pair tail): AV copy-out, denominator -> [128, 8] via a
#     DRAM bounce, 128-lane reciprocal, q-order row via a second bounce
#     (hwdge queues; plain strided APs so RAW deps track);
#     norm_finish (next pair, chunks 2/3): rank-1 f32r matmul broadcast +
#     one DVE multiply; odd heads hop to partition base 64 by a small DMA
#   - out[q,:]: two-pass projection, fc 0..2 accumulated to bf16 (ACT)
#     while the last norm chains drain, then fc 3 + bias (DVE) + pass-1
#     accumulator (Pool) + store

import numpy as np
import ml_dtypes

import concourse.bass as bass
import concourse.tile as tile
from concourse import bacc, mybir
from concourse.bass_utils import run_bass_kernel_spmd

F32 = mybir.dt.float32
BF16 = mybir.dt.bfloat16
I32 = mybir.dt.int32

H = 8
D = 64
E = 512
N_CORES = 8
SQ = 1024             # per-core query rows
MASK_BIAS = -1.25e8   # == -1e9 / sqrt(64), applied pre-softmax


def _emit(tc, t, SQ, SK, has_qbias):
    nc = tc.nc
    NQC = SQ // 128
    NKC = SK // 128           # key chunks
    QGS = 512                 # q group size for matmul free dim (1 PSUM bank)
    NQG = SQ // QGS
    LAG = max(1, min(4, NKC - 2))  # attn@V chunk lag inside a pair
    F32R = mybir.dt.float32r
    sub, mult, add = (mybir.AluOpType.subtract, mybir.AluOpType.mult,
                      mybir.AluOpType.add)

    with (
        tc.tile_pool(name="singles", bufs=1) as singles,
        tc.tile_pool(name="expp", bufs=4) as expp,
        tc.tile_pool(name="small", bufs=4) as small,
        tc.tile_pool(name="outp", bufs=2) as outp,
        tc.tile_pool(name="p_sc", bufs=2, space="PSUM") as p_sc,
        tc.tile_pool(name="p_av", bufs=2, space="PSUM") as p_av,
    ):
        # ---- persistent tensors, loaded directly in device layout ----
        qt = singles.tile([128, NQG, 4, QGS], BF16)  # query^T, group-major
        kt = singles.tile([128, NKC, 4, 128], BF16)  # key^T, chunk-major
        vt = singles.tile([128, NKC, H, D + 1], BF16)  # value + ones col
        xqa = singles.tile([128, NQG, 4, QGS], BF16)  # (Xq @ A)^T
        avt = singles.tile([128, 4, SQ], BF16)   # normalized (attn@V)^T
        out_acc = singles.tile([128, NQC, E], BF16)

        a2 = singles.tile([128, 128], BF16)       # blockdiag(A, A)
        nc.scalar.dma_start(a2, t["a2"][:])
        nc.sync.dma_start(qt[:], t["query"][:])
        kh = NKC // 2 + 1
        nc.sync.dma_start(kt[:, 0:kh], t["key"][:, 0:kh])
        nc.scalar.dma_start(kt[:, kh:NKC], t["key"][:, kh:NKC])
        nc.gpsimd.dma_start(vt[:], t["value"][:])
        m_sb = singles.tile([128, 4, E], BF16)
        nc.gpsimd.dma_start(m_sb, t["m2"][:])
        btot_rep = singles.tile([128, E], F32)
        nc.gpsimd.dma_start(btot_rep,
                            t["btot"][:][None, :].to_broadcast([128, E]))
        ones1f = singles.tile([1, D], F32)
        nc.vector.memset(ones1f, 1.0)
        ones1 = singles.tile([1, D], F32R)       # f32r needs a rounding producer
        nc.vector.tensor_copy(ones1, ones1f)

        # mask -> additive bias, [128, NKC] with k = kt*128 + p (host lays
        # the mask out p-major so this load is contiguous per partition)
        mask_i = singles.tile([128, NKC], I32)
        nc.gpsimd.dma_start(mask_i,
                            t["mask"][:].rearrange("(p kt) -> p kt", p=128))
        mask_f = singles.tile([128, NKC], F32)
        nc.vector.tensor_copy(mask_f, mask_i)
        mbias = singles.tile([128, NKC], F32)
        # (mask - 1) * (-MASK_BIAS):  mask=0 -> MASK_BIAS, mask=1 -> 0
        nc.vector.tensor_scalar(out=mbias, in0=mask_f, scalar1=1.0,
                                scalar2=-MASK_BIAS, op0=sub, op1=mult)

        # ---- blockdiag-A projection of Q^T. Pair p only reads column
        # fc=p, so fc=0 is emitted up front and the rest interleave into
        # pair-0's first chunks ----
        def emit_xqa(fc):
            for g in range(NQG):
                ps = p_sc.tile([128, QGS], F32, tag="sc", name=f"xq{fc}{g}")
                nc.tensor.matmul(ps, lhsT=a2, rhs=qt[:, g, fc, :],
                                 start=True, stop=True)
                if (fc + g) % 2 == 0:
                    nc.vector.tensor_copy(xqa[:, g, fc, :], ps)
                else:
                    nc.scalar.activation(out=xqa[:, g, fc, :], in_=ps,
                                         func=mybir.ActivationFunctionType.Copy)
        emit_xqa(0)

        # ---- optional exact bq correction: per-(h,k) additive bias ----
        if has_qbias:
            w2 = singles.tile([128, 1], BF16)
            nc.sync.dma_start(w2, t["w2"][:])
            hbias = []
            for h in range(H):
                bp, fc = 64 * (h % 2), h // 2
                row = small.tile([1, SK], F32, tag="hb_row")
                for g in range(SK // 512):
                    ps = p_sc.tile([128, 512], F32, tag="sc")
                    nc.tensor.matmul(ps[0:1, :], lhsT=w2[bp:bp + 64, :],
                                     rhs=kt[bp:bp + 64, 4 * g:4 * g + 4, fc, :],
                                     start=True, stop=True)
                    nc.vector.tensor_copy(row[:, g * 512:(g + 1) * 512],
                                          ps[0:1, :])
                hb = singles.tile([128, NKC], F32, name=f"hbias{h}")
                nc.gpsimd.dma_start(hb, row[0, :].rearrange("(kt p) -> p kt",
                                                            p=128))
                nc.vector.tensor_tensor(out=hb, in0=hb, in1=mbias, op=add)
                hbias.append(hb)
        else:
            hbias = [mbias] * H

        # ---- main loop ----
        ex = {}       # h -> [128, NKC, SQ] bf16 exp tiles
        av_ps = {}    # h -> [65, SQ] f32 PSUM accumulator
        norm_st = {}  # h -> (avsb, rrow) from norm_start

        def sc_exp(h, c):
            bp, fc = 64 * (h % 2), h // 2
            ps = p_sc.tile([128, SQ], F32, tag="sc")
            for g in range(NQG):
                gsl = slice(g * QGS, (g + 1) * QGS)
                nc.tensor.matmul(ps[:, gsl],
                                 lhsT=kt[bp:bp + 64, c, fc, :],
                                 rhs=xqa[bp:bp + 64, g, fc, :],
                                 start=True, stop=True)
            nc.scalar.activation(out=ex[h][:, c, :], in_=ps,
                                 func=mybir.ActivationFunctionType.Exp,
                                 bias=hbias[h][:, c:c + 1], scale=1.0)

        def av_mm(h, c):
            for g in range(NQG):
                gsl = slice(g * QGS, (g + 1) * QGS)
                nc.tensor.matmul(av_ps[h][:, gsl], lhsT=vt[:, c, h, :],
                                 rhs=ex[h][:, c, gsl],
                                 start=(c == 0), stop=(c == NKC - 1))

        def norm_start(h, q):
            # Copy AV+denominator out of PSUM (frees the accumulator).
            # Denominator row -> [128, 8] via a DRAM bounce so the
            # reciprocal runs on all DVE lanes; second bounce restores
            # q-order. The two heads of a pair use different DMA queues
            # so their chains run concurrently.
            avsb = small.tile([D + 1, SQ], F32, tag="avsb", name=f"avsb{h}")
            nc.vector.tensor_copy(avsb, av_ps[h])
            del av_ps[h]
            q.dma_start(t["dscr"][h, :], avsb[D:D + 1, :])
            dn = small.tile([128, SQ // 128], F32, tag="dn", name=f"dn{h}")
            q.dma_start(dn, t["dscr"][h, :].rearrange("(f p) -> p f", p=128))
            rc = small.tile([128, SQ // 128], F32, tag="rc", name=f"rc{h}")
            nc.vector.reciprocal(rc, dn)
            q.dma_start(t["rscr"][h, :].rearrange("(f p) -> p f", p=128), rc)
            rrow = small.tile([1, SQ], F32, tag="rrow", name=f"rrow{h}")
            q.dma_start(rrow, t["rscr"][h:h + 1, :])
            norm_st[h] = (avsb, rrow)

        def norm_finish(h):
            # rank-1 broadcast of 1/den across partitions, one multiply
            fc = h // 2
            avsb, rrow = norm_st.pop(h)
            pb = p_av.tile([D, SQ], F32, tag="av")
            for g in range(NQG):
                gsl = slice(g * QGS, (g + 1) * QGS)
                nc.tensor.matmul(pb[:, gsl], lhsT=ones1,
                                 rhs=rrow[0:1, gsl].bitcast(F32R),
                                 start=True, stop=True)
            if h % 2 == 0:
                nc.vector.tensor_tensor(out=avt[0:D, fc, :],
                                        in0=avsb[0:D, :], in1=pb, op=mult)
            else:
                avtmp = small.tile([64, SQ], BF16, tag="avtmp")
                nc.vector.tensor_tensor(out=avtmp, in0=avsb[0:D, :],
                                        in1=pb, op=mult)
                nc.gpsimd.dma_start(avt[64:64 + D, fc, :], avtmp)

        for p in range(H // 2):
            h0, h1 = 2 * p, 2 * p + 1
            ex[h0] = expp.tile([128, NKC, SQ], BF16, tag="exp", name=f"ex{h0}")
            ex[h1] = expp.tile([128, NKC, SQ], BF16, tag="exp", name=f"ex{h1}")
            for c in range(NKC):
                if p == 0 and 1 <= c <= 3:
                    emit_xqa(c)
                sc_exp(h0, c)
                sc_exp(h1, c)
                if p > 0 and c == 2:
                    norm_finish(h0 - 2)
                if p > 0 and c == LAG:
                    norm_finish(h1 - 2)
                if c >= LAG:
                    if c == LAG:
                        av_ps[h0] = p_av.tile([D + 1, SQ], F32, tag="av",
                                              name=f"av{h0}")
                        av_ps[h1] = p_av.tile([D + 1, SQ], F32, tag="av",
                                              name=f"av{h1}")
                    av_mm(h0, c - LAG)
                    av_mm(h1, c - LAG)
            # pair tail: per-head trailing AV then its norm chain, so each
            # chain starts as early as possible on its own DMA queue
            for cc in range(NKC - LAG, NKC):
                av_mm(h0, cc)
            norm_start(h0, nc.sync)
            for cc in range(NKC - LAG, NKC):
                av_mm(h1, cc)
            norm_start(h1, nc.gpsimd)

        # ---- output projection pass 1 (fc 0..2) while the last pair's
        # norm chains drain ----
        for q_i in range(SQ // 128):
            qs = slice(q_i * 128, (q_i + 1) * 128)
            ps = p_sc.tile([128, E], F32, tag="sc")
            for j in range(3):
                nc.tensor.matmul(ps, lhsT=avt[:, j, qs], rhs=m_sb[:, j, :],
                                 start=(j == 0), stop=(j == 2))
            nc.scalar.activation(out=out_acc[:, q_i, :], in_=ps,
                                 func=mybir.ActivationFunctionType.Copy)
        norm_finish(H - 2)
        norm_finish(H - 1)
        # ---- pass 2: last head pair + bias (DVE) + pass-1 acc (Pool) ----
        for q_i in range(SQ // 128):
            qs = slice(q_i * 128, (q_i + 1) * 128)
            ps = p_sc.tile([128, E], F32, tag="sc")
            nc.tensor.matmul(ps, lhsT=avt[:, 3, qs], rhs=m_sb[:, 3, :],
                             start=True, stop=True)
            ob = outp.tile([128, E], F32, tag="ob")
            nc.vector.tensor_tensor(out=ob, in0=ps, in1=btot_rep, op=add)
            ob2 = outp.tile([128, E], F32, tag="ob2")
            nc.gpsimd.tensor_tensor(out=ob2, in0=ob, in1=out_acc[:, q_i, :],
                                    op=add)
            nc.sync.dma_start(t["out"][qs, :], ob2)


def build_module(SQ, SK, has_qbias):
    NKC = SK // 128
    nc = bacc.Bacc()
    t = {
        "query": nc.dram_tensor("query", [128, SQ // 512, 4, 512], BF16,
                                kind="ExternalInput"),
        "key": nc.dram_tensor("key", [128, NKC, 4, 128], BF16,
                              kind="ExternalInput"),
        "value": nc.dram_tensor("value", [128, NKC, H, D + 1], BF16,
                                kind="ExternalInput"),
        "mask": nc.dram_tensor("mask", [SK], I32, kind="ExternalInput"),
        "a2": nc.dram_tensor("a2", [128, 128], BF16, kind="ExternalInput"),
        "m2": nc.dram_tensor("m2", [128, 4, E], BF16, kind="ExternalInput"),
        "btot": nc.dram_tensor("btot", [E], F32, kind="ExternalInput"),
        "out": nc.dram_tensor("out", [SQ, E], F32, kind="ExternalOutput"),
        "dscr": nc.dram_tensor("dscr", [H, SQ], F32, kind="Internal"),
        "rscr": nc.dram_tensor("rscr", [H, SQ], F32, kind="Internal"),
    }
    if has_qbias:
        t["w2"] = nc.dram_tensor("w2", [128, 1], BF16, kind="ExternalInput")
    with tile.TileContext(nc) as tc:
        _emit(tc, t, SQ, SK, has_qbias)
    nc.compile()
    return nc


_MODULE_CACHE = {}


def _get_module(SQ, SK, has_qbias):
    key = (SQ, SK, has_qbias)
    if key not in _MODULE_CACHE:
        _MODULE_CACHE[key] = build_module(SQ, SK, has_qbias)
    return _MODULE_CACHE[key]


def _fold_weights(Wq, Wk, Wv, Wo, bv, bo):
    Wq, Wk, Wv, Wo = (np.asarray(w, np.float64) for w in (Wq, Wk, Wv, Wo))
    A = (Wq.T @ Wk) / np.sqrt(np.float64(D))
    a2 = np.zeros((128, 128), np.float64)     # blockdiag(A, A)
    a2[:D, :D] = A
    a2[D:, D:] = A
    a2 = a2.astype(ml_dtypes.bfloat16)
    Ms = [Wv.T @ Wo[:, h * D:(h + 1) * D].T for h in range(H)]
    # head-pair packing: head h at partitions 64*(h%2) .. +64, free slot h//2
    m2 = np.zeros((128, 4, E), np.float64)
    for h in range(H):
        m2[64 * (h % 2):64 * (h % 2) + D, h // 2, :] = Ms[h]
    m2 = m2.astype(ml_dtypes.bfloat16)
    btot = (np.asarray(bo, np.float64)
            + Wo @ np.tile(np.asarray(bv, np.float64), H)).astype(np.float32)
    return a2, m2, btot


def _run(inputs, trace=False):
    query = np.asarray(inputs["query"], np.float32)
    key = np.asarray(inputs["key"], np.float32)
    value = np.asarray(inputs["value"], np.float32)
    mask = np.asarray(inputs["mask"])
    a2, m2, btot = _fold_weights(inputs["Wq"], inputs["Wk"], inputs["Wv"],
                                 inputs["Wo"], inputs["bv"], inputs["bo"])
    bq = np.asarray(inputs["bq"], np.float64)
    has_qbias = bool(np.any(bq != 0))
    w2 = None
    if has_qbias:
        w2v = (np.asarray(inputs["Wk"], np.float64).T @ bq) / np.sqrt(float(D))
        w2 = np.concatenate([w2v, w2v]).reshape(128, 1).astype(ml_dtypes.bfloat16)

    n_batch, S = query.shape[0], query.shape[1]
    sq = S // 2

    # ---- key compaction: drop masked keys, pad to a common SKC ----
    idxs = [np.flatnonzero(mask[n, 0, 0, :] != 0) for n in range(n_batch)]
    maxk = max(int(ix.size) for ix in idxs)
    SKC = max(256, -(-maxk // 128) * 128)
    NKC = SKC // 128
    key_c = np.zeros((n_batch, SKC, E), ml_dtypes.bfloat16)
    val_c = np.zeros((n_batch, SKC, E), ml_dtypes.bfloat16)
    msk_c = np.zeros((n_batch, SKC), np.int32)
    for n, ix in enumerate(idxs):
        key_c[n, :ix.size] = key[n][ix]
        val_c[n, :ix.size] = value[n][ix]
        msk_c[n, :ix.size] = 1
    # device layouts: q/k as [128, 4, s] transposes, v padded-head + ones
    # column, mask p-major
    # k_h[p, c, fc, j] = K[c*128+j, fc*128+p]
    kt_h = np.ascontiguousarray(
        key_c.reshape(n_batch, NKC, 128, 4, 128).transpose(0, 4, 1, 3, 2))
    vt_h = np.ones((n_batch, 128, NKC, H, D + 1), ml_dtypes.bfloat16)
    vt_h[..., :D] = val_c.reshape(n_batch, NKC, 128, H, D).transpose(
        0, 2, 1, 3, 4)
    msk_t = np.ascontiguousarray(
        msk_c.reshape(n_batch, NKC, 128).transpose(0, 2, 1)
    ).reshape(n_batch, SKC)
    query_b = query.astype(ml_dtypes.bfloat16)

    nc = _get_module(sq, SKC, has_qbias)

    in_maps = []
    for c in range(N_CORES):
        n, qh = divmod(c, 2)
        # q_h[p, g, fc, j] = Xq[g*512+j, fc*128+p]
        qt_h = np.ascontiguousarray(
            query_b[n, qh * sq:(qh + 1) * sq, :]
            .reshape(sq // 512, 512, 4, 128).transpose(3, 0, 2, 1))
        m = {
            "query": qt_h,
            "key": kt_h[n],
            "value": vt_h[n],
            "mask": msk_t[n],
            "a2": a2, "m2": m2, "btot": btot,
        }
        if has_qbias:
            m["w2"] = w2
        in_maps.append(m)

    res = run_bass_kernel_spmd(nc, in_maps, core_ids=list(range(N_CORES)),
                               trace=trace)
    out = np.empty((n_batch, S, E), np.float32)
    for c, r in enumerate(res.results):
        n, qh = divmod(c, 2)
        out[n, qh * sq:(qh + 1) * sq, :] = r["out"]
    return out, res


def kernel(**inputs) -> np.ndarray:
    out, _ = _run(inputs, trace=False)
    return out


# revision 33
# speedup vs baseline: 1.1320x; 1.1320x over previous
# Multi-head attention (N=4, S=2048, E=512, H=8, D=64) on 8 NeuronCores.
#
# Sharding: core c -> (batch n = c//2, query half qh = c%2). Each core
# computes attention for its 1024 query rows against the (compacted) keys of
# its batch, all 8 heads, and the full output projection for its rows, so
# outputs are disjoint and no collectives are needed.
#
# Key compaction: the mask is per-(batch, key) broadcast over queries/heads,
# and masked keys get exp(-1.25e8) == 0 exactly in f32, contributing nothing
# to the numerator or denominator. So keys with mask==0 are dropped on the
# host; all batches pad to a common chunk-multiple length SKC with pad keys
# biased to -inf. This halves score/exp/attn@V work for ~50% masks.
#
# Host-side prep (pure weight algebra / data layout, done once):
#   A  = Wq^T @ Wk / 8          scores = (Xq @ A) @ Xk^T  (raw K, one proj)
#   M_h = Wv^T @ Wo[:, h]^T     out += (attn @ Xv_h) @ M_h (Wv applied post)
#   btot = bo + Wo @ tile(bv,8) exact because attention rows sum to 1
#   bk-term cancels in softmax (constant over k); bq-term handled via a
#   per-k bias correction (zero for this problem's inputs).
#   Q/K/V are cast to bf16 and pre-laid-out in the SBUF-resident shapes
#   ([e, s] transposes, V with its ones column), so on device they are
#   plain contiguous DMAs with chunk-granular dependencies.
#
# Device per core (head pair p owns heads 2p, 2p+1 at partition bases 0/64):
#   - xqa = blockdiag(A,A)-projection of Q^T per head pair (128-contraction)
#   - per (head, k-chunk): scores^T[k,q] on PE -> exp on ACT straight from
#     PSUM with per-partition mask bias -> bf16 ex tiles
#   - attn@V accumulates per chunk at 4-chunk lag (V carries a ones column
#     so softmax denominators ride along as PSUM row 64)
#   - normalization is split so no engine queue ever waits on its DMAs:
#     norm_start (pair tail): AV copy-out, denominator -> [128, 8] via a
#     DRAM bounce, 128-lane reciprocal, q-order row via a second bounce
#     (hwdge queues; plain strided APs so RAW deps track);
#     norm_finish (next pair, chunks 2/3): rank-1 f32r matmul broadcast +
#     one DVE multiply; odd heads hop to partition base 64 by a small DMA
#   - out[q,:]: two-pass projection, fc 0..2 accumulated to bf16 (ACT)
#     while the last norm chains drain, then fc 3 + bias (DVE) + pass-1
#     accumulator (Pool) + store

import numpy as np
import ml_dtypes

import concourse.bass as bass
import concourse.tile as tile
from concourse import bacc, mybir
from concourse.bass_utils import run_bass_kernel_spmd

F32 = mybir.dt.float32
BF16 = mybir.dt.bfloat16
I32 = mybir.dt.int32

H = 8
D = 64
E = 512
N_CORES = 8
SQ = 1024             # per-core query rows
MASK_BIAS = -1.25e8   # == -1e9 / sqrt(64), applied pre-softmax


def _emit(tc, t, SQ, SK, has_qbias):
    nc = tc.nc
    NQC = SQ // 128
    NKC = SK // 128           # key chunks
    QGS = 512                 # q group size for matmul free dim (1 PSUM bank)
    NQG = SQ // QGS
    LAG = max(1, min(4, NKC - 2))  # attn@V chunk lag inside a pair
    F32R = mybir.dt.float32r
    sub, mult, add = (mybir.AluOpType.subtract, mybir.AluOpType.mult,
                      mybir.AluOpType.add)

    with (
        tc.tile_pool(name="singles", bufs=1) as singles,
        tc.tile_pool(name="expp", bufs=4) as expp,
        tc.tile_pool(name="small", bufs=4) as small,
        tc.tile_pool(name="outp", bufs=2) as outp,
        tc.tile_pool(name="p_sc", bufs=2, space="PSUM") as p_sc,
        tc.tile_pool(name="p_av", bufs=2, space="PSUM") as p_av,
    ):
        # ---- persistent tensors, loaded directly in device layout ----
        qt = singles.tile([128, NQG, 4, QGS], BF16)  # query^T, group-major
        kt = singles.tile([128, NKC, 4, 128], BF16)  # key^T, chunk-major
        vt = singles.tile([128, NKC, H, D + 1], BF16)  # value + ones col
        xqa = singles.tile([128, NQG, 4, QGS], BF16)  # (Xq @ A)^T
        avt = singles.tile([128, 4, SQ], BF16)   # normalized (attn@V)^T
        out_acc = singles.tile([128, NQC, E], BF16)

        a2 = singles.tile([128, 128], BF16)       # blockdiag(A, A)
        nc.scalar.dma_start(a2, t["a2"][:])
        nc.sync.dma_start(qt[:], t["query"][:])
        kh = NKC // 2 + 1
        nc.sync.dma_start(kt[:, 0:kh], t["key"][:, 0:kh])
        nc.scalar.dma_start(kt[:, kh:NKC], t["key"][:, kh:NKC])
        nc.gpsimd.dma_start(vt[:], t["value"][:])
        m_sb = singles.tile([128, 4, E], BF16)
        nc.gpsimd.dma_start(m_sb, t["m2"][:])
        btot_rep = singles.tile([128, E], F32)
        nc.gpsimd.dma_start(btot_rep,
                            t["btot"][:][None, :].to_broadcast([128, E]))
        ones1f = singles.tile([1, D], F32)
        nc.vector.memset(ones1f, 1.0)
        ones1 = singles.tile([1, D], F32R)       # f32r needs a rounding producer
        nc.vector.tensor_copy(ones1, ones1f)
        ones65 = singles.tile([D + 1, 1], F32)   # rank-1 rhs at partition D
        nc.vector.memset(ones65, 1.0)

        # mask -> additive bias, [128, NKC] with k = kt*128 + p (host lays
        # the mask out p-major so this load is contiguous per partition)
        mask_i = singles.tile([128, NKC], I32)
        nc.gpsimd.dma_start(mask_i,
                            t["mask"][:].rearrange("(p kt) -> p kt", p=128))
        mask_f = singles.tile([128, NKC], F32)
        nc.vector.tensor_copy(mask_f, mask_i)
        mbias = singles.tile([128, NKC], F32)
        # (mask - 1) * (-MASK_BIAS):  mask=0 -> MASK_BIAS, mask=1 -> 0
        nc.vector.tensor_scalar(out=mbias, in0=mask_f, scalar1=1.0,
                                scalar2=-MASK_BIAS, op0=sub, op1=mult)

        # ---- blockdiag-A projection of Q^T. Pair p only reads column
        # fc=p, so fc=0 is emitted up front and the rest interleave into
        # pair-0's first chunks ----
        def emit_xqa(fc):
            for g in range(NQG):
                ps = p_sc.tile([128, QGS], F32, tag="sc", name=f"xq{fc}{g}")
                nc.tensor.matmul(ps, lhsT=a2, rhs=qt[:, g, fc, :],
                                 start=True, stop=True)
                if (fc + g) % 2 == 0:
                    nc.vector.tensor_copy(xqa[:, g, fc, :], ps)
                else:
                    nc.scalar.activation(out=xqa[:, g, fc, :], in_=ps,
                                         func=mybir.ActivationFunctionType.Copy)
        emit_xqa(0)

        # ---- optional exact bq correction: per-(h,k) additive bias ----
        if has_qbias:
            w2 = singles.tile([128, 1], BF16)
            nc.sync.dma_start(w2, t["w2"][:])
            hbias = []
            for h in range(H):
                bp, fc = 64 * (h % 2), h // 2
                row = small.tile([1, SK], F32, tag="hb_row")
                for g in range(SK // 512):
                    ps = p_sc.tile([128, 512], F32, tag="sc")
                    nc.tensor.matmul(ps[0:1, :], lhsT=w2[bp:bp + 64, :],
                                     rhs=kt[bp:bp + 64, 4 * g:4 * g + 4, fc, :],
                                     start=True, stop=True)
                    nc.vector.tensor_copy(row[:, g * 512:(g + 1) * 512],
                                          ps[0:1, :])
                hb = singles.tile([128, NKC], F32, name=f"hbias{h}")
                nc.gpsimd.dma_start(hb, row[0, :].rearrange("(kt p) -> p kt",
                                                            p=128))
                nc.vector.tensor_tensor(out=hb, in0=hb, in1=mbias, op=add)
                hbias.append(hb)
        else:
            hbias = [mbias] * H

        # ---- main loop ----
        ex = {}       # h -> [128, NKC, SQ] bf16 exp tiles
        av_ps = {}    # h -> [65, SQ] f32 PSUM accumulator
        norm_st = {}  # h -> (avsb, rrow) from norm_start

        def sc_exp(h, c):
            bp, fc = 64 * (h % 2), h // 2
            ps = p_sc.tile([128, SQ], F32, tag="sc")
            for g in range(NQG):
                gsl = slice(g * QGS, (g + 1) * QGS)
                nc.tensor.matmul(ps[:, gsl],
                                 lhsT=kt[bp:bp + 64, c, fc, :],
                                 rhs=xqa[bp:bp + 64, g, fc, :],
                                 start=True, stop=True)
            nc.scalar.activation(out=ex[h][:, c, :], in_=ps,
                                 func=mybir.ActivationFunctionType.Exp,
                                 bias=hbias[h][:, c:c + 1], scale=1.0)

        def av_mm(h, c):
            for g in range(NQG):
                gsl = slice(g * QGS, (g + 1) * QGS)
                nc.tensor.matmul(av_ps[h][:, gsl], lhsT=vt[:, c, h, :],
                                 rhs=ex[h][:, c, gsl],
                                 start=(c == 0), stop=(c == NKC - 1))

        def norm_start(h, q):
            # Copy AV+denominator out of PSUM (frees the accumulator).
            # Denominator row -> [128, 8] with eight trivial rank-1 PE
            # matmuls (no DMA latency), 128-lane reciprocal straight from
            # PSUM, then ONE DRAM bounce restores q-order for the
            # broadcast matmul in norm_finish.
            avsb = small.tile([D + 1, SQ], F32, tag="avsb", name=f"avsb{h}")
            nc.vector.tensor_copy(avsb, av_ps[h])
            del av_ps[h]
            dnp = p_av.tile([128, SQ // 128], F32, tag="av", name=f"dnp{h}")
            for f in range(SQ // 128):
                nc.tensor.matmul(dnp[:, f:f + 1],
                                 lhsT=avsb[D:D + 1, f * 128:(f + 1) * 128],
                                 rhs=ones65[D:D + 1, :],
                                 start=True, stop=True)
            rc = small.tile([128, SQ // 128], F32, tag="rc", name=f"rc{h}")
            nc.vector.reciprocal(rc, dnp)
            q.dma_start(t["rscr"][h, :].rearrange("(f p) -> p f", p=128), rc)
            rrow = small.tile([1, SQ], F32, tag="rrow", name=f"rrow{h}")
            q.dma_start(rrow, t["rscr"][h:h + 1, :])
            norm_st[h] = (avsb, rrow)

        def norm_finish(h):
            # rank-1 broadcast of 1/den across partitions, one multiply
            fc = h // 2
            avsb, rrow = norm_st.pop(h)
            pb = p_av.tile([D, SQ], F32, tag="av")
            for g in range(NQG):
                gsl = slice(g * QGS, (g + 1) * QGS)
                nc.tensor.matmul(pb[:, gsl], lhsT=ones1,
                                 rhs=rrow[0:1, gsl].bitcast(F32R),
                                 start=True, stop=True)
            if h % 2 == 0:
                nc.vector.tensor_tensor(out=avt[0:D, fc, :],
                                        in0=avsb[0:D, :], in1=pb, op=mult)
            else:
                avtmp = small.tile([64, SQ], BF16, tag="avtmp")
                nc.vector.tensor_tensor(out=avtmp, in0=avsb[0:D, :],
                                        in1=pb, op=mult)
                nc.gpsimd.dma_start(avt[64:64 + D, fc, :], avtmp)

        for p in range(H // 2):
            h0, h1 = 2 * p, 2 * p + 1
            ex[h0] = expp.tile([128, NKC, SQ], BF16, tag="exp", name=f"ex{h0}")
            ex[h1] = expp.tile([128, NKC, SQ], BF16, tag="exp", name=f"ex{h1}")
            for c in range(NKC):
                if p == 0 and 1 <= c <= 3:
                    emit_xqa(c)
                sc_exp(h0, c)
                sc_exp(h1, c)
                if p > 0 and c == 2:
                    norm_finish(h0 - 2)
                if p > 0 and c == LAG:
                    norm_finish(h1 - 2)
                if c >= LAG:
                    if c == LAG:
                        av_ps[h0] = p_av.tile([D + 1, SQ], F32, tag="av",
                                              name=f"av{h0}")
                        av_ps[h1] = p_av.tile([D + 1, SQ], F32, tag="av",
                                              name=f"av{h1}")
                    av_mm(h0, c - LAG)
                    av_mm(h1, c - LAG)
            # pair tail: per-head trailing AV then its norm chain, so each
            # chain starts as early as possible on its own DMA queue
            for cc in range(NKC - LAG, NKC):
                av_mm(h0, cc)
            for cc in range(NKC - LAG, NKC):
                av_mm(h1, cc)
            norm_start(h0, nc.sync)
            norm_start(h1, nc.sync)

        # ---- output projection pass 1 (fc 0..2) while the last pair's
        # norm chains drain ----
        for q_i in range(SQ // 128):
            qs = slice(q_i * 128, (q_i + 1) * 128)
            ps = p_sc.tile([128, E], F32, tag="sc")
            for j in range(3):
                nc.tensor.matmul(ps, lhsT=avt[:, j, qs], rhs=m_sb[:, j, :],
                                 start=(j == 0), stop=(j == 2))
            nc.scalar.activation(out=out_acc[:, q_i, :], in_=ps,
                                 func=mybir.ActivationFunctionType.Copy)
        norm_finish(H - 2)
        norm_finish(H - 1)
        # ---- pass 2: last head pair + bias (DVE) + pass-1 acc (Pool) ----
        for q_i in range(SQ // 128):
            qs = slice(q_i * 128, (q_i + 1) * 128)
            ps = p_sc.tile([128, E], F32, tag="sc")
            nc.tensor.matmul(ps, lhsT=avt[:, 3, qs], rhs=m_sb[:, 3, :],
                             start=True, stop=True)
            ob = outp.tile([128, E], F32, tag="ob")
            nc.vector.tensor_tensor(out=ob, in0=ps, in1=btot_rep, op=add)
            ob2 = outp.tile([128, E], F32, tag="ob2")
            nc.gpsimd.tensor_tensor(out=ob2, in0=ob, in1=out_acc[:, q_i, :],
                                    op=add)
            nc.sync.dma_start(t["out"][qs, :], ob2)


def build_module(SQ, SK, has_qbias):
    NKC = SK // 128
    nc = bacc.Bacc()
    t = {
        "query": nc.dram_tensor("query", [128, SQ // 512, 4, 512], BF16,
                                kind="ExternalInput"),
        "key": nc.dram_tensor("key", [128, NKC, 4, 128], BF16,
                              kind="ExternalInput"),
        "value": nc.dram_tensor("value", [128, NKC, H, D + 1], BF16,
                                kind="ExternalInput"),
        "mask": nc.dram_tensor("mask", [SK], I32, kind="ExternalInput"),
        "a2": nc.dram_tensor("a2", [128, 128], BF16, kind="ExternalInput"),
        "m2": nc.dram_tensor("m2", [128, 4, E], BF16, kind="ExternalInput"),
        "btot": nc.dram_tensor("btot", [E], F32, kind="ExternalInput"),
        "out": nc.dram_tensor("out", [SQ, E], F32, kind="ExternalOutput"),
        "rscr": nc.dram_tensor("rscr", [H, SQ], F32, kind="Internal"),
    }
    if has_qbias:
        t["w2"] = nc.dram_tensor("w2", [128, 1], BF16, kind="ExternalInput")
    with tile.TileContext(nc) as tc:
        _emit(tc, t, SQ, SK, has_qbias)
    nc.compile()
    return nc


_MODULE_CACHE = {}


def _get_module(SQ, SK, has_qbias):
    key = (SQ, SK, has_qbias)
    if key not in _MODULE_CACHE:
        _MODULE_CACHE[key] = build_module(SQ, SK, has_qbias)
    return _MODULE_CACHE[key]


def _fold_weights(Wq, Wk, Wv, Wo, bv, bo):
    Wq, Wk, Wv, Wo = (np.asarray(w, np.float64) for w in (Wq, Wk, Wv, Wo))
    A = (Wq.T @ Wk) / np.sqrt(np.float64(D))
    a2 = np.zeros((128, 128), np.float64)     # blockdiag(A, A)
    a2[:D, :D] = A
    a2[D:, D:] = A
    a2 = a2.astype(ml_dtypes.bfloat16)
    Ms = [Wv.T @ Wo[:, h * D:(h + 1) * D].T for h in range(H)]
    # head-pair packing: head h at partitions 64*(h%2) .. +64, free slot h//2
    m2 = np.zeros((128, 4, E), np.float64)
    for h in range(H):
        m2[64 * (h % 2):64 * (h % 2) + D, h // 2, :] = Ms[h]
    m2 = m2.astype(ml_dtypes.bfloat16)
    btot = (np.asarray(bo, np.float64)
            + Wo @ np.tile(np.asarray(bv, np.float64), H)).astype(np.float32)
    return a2, m2, btot


def _run(inputs, trace=False):
    query = np.asarray(inputs["query"], np.float32)
    key = np.asarray(inputs["key"], np.float32)
    value = np.asarray(inputs["value"], np.float32)
    mask = np.asarray(inputs["mask"])
    a2, m2, btot = _fold_weights(inputs["Wq"], inputs["Wk"], inputs["Wv"],
                                 inputs["Wo"], inputs["bv"], inputs["bo"])
    bq = np.asarray(inputs["bq"], np.float64)
    has_qbias = bool(np.any(bq != 0))
    w2 = None
    if has_qbias:
        w2v = (np.asarray(inputs["Wk"], np.float64).T @ bq) / np.sqrt(float(D))
        w2 = np.concatenate([w2v, w2v]).reshape(128, 1).astype(ml_dtypes.bfloat16)

    n_batch, S = query.shape[0], query.shape[1]
    sq = S // 2

    # ---- key compaction: drop masked keys, pad to a common SKC ----
    idxs = [np.flatnonzero(mask[n, 0, 0, :] != 0) for n in range(n_batch)]
    maxk = max(int(ix.size) for ix in idxs)
    SKC = max(256, -(-maxk // 128) * 128)
    NKC = SKC // 128
    key_c = np.zeros((n_batch, SKC, E), ml_dtypes.bfloat16)
    val_c = np.zeros((n_batch, SKC, E), ml_dtypes.bfloat16)
    msk_c = np.zeros((n_batch, SKC), np.int32)
    for n, ix in enumerate(idxs):
        key_c[n, :ix.size] = key[n][ix]
        val_c[n, :ix.size] = value[n][ix]
        msk_c[n, :ix.size] = 1
    # device layouts: q/k as [128, 4, s] transposes, v padded-head + ones
    # column, mask p-major
    # k_h[p, c, fc, j] = K[c*128+j, fc*128+p]
    kt_h = np.ascontiguousarray(
        key_c.reshape(n_batch, NKC, 128, 4, 128).transpose(0, 4, 1, 3, 2))
    vt_h = np.ones((n_batch, 128, NKC, H, D + 1), ml_dtypes.bfloat16)
    vt_h[..., :D] = val_c.reshape(n_batch, NKC, 128, H, D).transpose(
        0, 2, 1, 3, 4)
    msk_t = np.ascontiguousarray(
        msk_c.reshape(n_batch, NKC, 128).transpose(0, 2, 1)
    ).reshape(n_batch, SKC)
    query_b = query.astype(ml_dtypes.bfloat16)

    nc = _get_module(sq, SKC, has_qbias)

    in_maps = []
    for c in range(N_CORES):
        n, qh = divmod(c, 2)
        # q_h[p, g, fc, j] = Xq[g*512+j, fc*128+p]
        qt_h = np.ascontiguousarray(
            query_b[n, qh * sq:(qh + 1) * sq, :]
            .reshape(sq // 512, 512, 4, 128).transpose(3, 0, 2, 1))
        m = {
            "query": qt_h,
            "key": kt_h[n],
            "value": vt_h[n],
            "mask": msk_t[n],
            "a2": a2, "m2": m2, "btot": btot,
        }
        if has_qbias:
            m["w2"] = w2
        in_maps.append(m)

    res = run_bass_kernel_spmd(nc, in_maps, core_ids=list(range(N_CORES)),
                               trace=trace)
    out = np.empty((n_batch, S, E), np.float32)
    for c, r in enumerate(res.results):
        n, qh = divmod(c, 2)
        out[n, qh * sq:(qh + 1) * sq, :] = r["out"]
    return out, res


def kernel(**inputs) -> np.ndarray:
    out, _ = _run(inputs, trace=False)
    return out
